# revision 10
# baseline (speedup 1.0000x reference)
"""Trainium2 Bass kernel for nn_Block_41077067219413.

Reference computation (B=2048, D=dim_in=4096, J=dim_out=4096):
    xf = x.astype(f32)                 # (B, D) in {0,1}
    mf = masks.astype(f32)             # (D, J) in {0,1}
    sums = xf @ mf + (1-xf) @ (1-mf)   # XNOR popcount over D
    out  = sums > thresholds[None, :]  # (B, J) bool

Identity: with x' = 2x-1 and m' = 2m-1 (both in {-1,+1}),
    A'[j,b] = sum_k m'[k,j] * x'[b,k] = 2*sums - D
    out     = A' > 2*th - D

Both operands are host-encoded as fp8e4 bytes (+1 = 0x38, -1 = 0xB8) and
host-tiled so every DMA is 128 partitions x 1024 contiguous bytes.  The
device runs one fp8 DoubleRow GEMM per core with no transposes, rowsum,
or threshold folding.  Work is sharded 2 (batch) x 4 (dim_out): each
core computes out_shard [1024 j, 1024 b] = m'^T @ x' with j on PSUM
partitions, so the threshold compare is a per-partition-scalar is_gt.

Schedule: a few warm-up matmuls on a zeroed tile un-throttle the PE
clock while the first tiles stream in, then 4 passes of (j-half,
b-half), 64 DR matmuls each, alternating two 4-bank PSUM sets so
epilogues overlap the next pass.  Input DMAs are spread across the
gpsimd/vector (masks) and scalar/sync (x) queues, pass-1 tiles first.
"""

import numpy as np

B, D, J = 2048, 4096, 4096
NCORES = 8
BS, JS = 2, 4             # batch-shards x j-shards
BL = B // BS              # 1024 batch rows per core
JL = J // JS              # 1024 output cols per core
P = 128
KP = D // 256             # 16 k-pair tiles (256 contraction each)
JT = JL // P              # 8 j-tiles of 128
BC = 512                  # batch free-dim chunk (one PSUM bank)
NBC = BL // BC            # 2 batch chunks
JH = JT // 2              # 4 j-tiles per pass
NWARM = 7

_cache = {}


def _build():
    import concourse.bacc as bacc
    import concourse.mybir as mybir
    import concourse.tile as tile

    dt = mybir.dt
    f8 = dt.float8e4
    f32 = dt.float32
    ALU = mybir.AluOpType
    AF = mybir.ActivationFunctionType
    DR = mybir.MatmulPerfMode.DoubleRow

    nc = bacc.Bacc("TRN2", target_bir_lowering=False, debug=False,
                   num_devices=NCORES)

    # host-tiled fp8 bytes: row r = (chunk*KP + kp)*128 + ki holds the
    # 1024 bytes [ko=0 512 | ko=1 512] for that partition
    x_d = nc.dram_tensor("xp", [NBC * KP * P, 1024], dt.uint8,
                         kind="ExternalInput")
    m_d = nc.dram_tensor("mp", [2 * KP * P, 1024], dt.uint8,
                         kind="ExternalInput")
    c_d = nc.dram_tensor("cth", [P, JT], f32, kind="ExternalInput")
    # -(c+1) for the Sign-based epilogue on the Activation engine
    cn_d = nc.dram_tensor("cng", [P, JT], f32, kind="ExternalInput")
    o_d = nc.dram_tensor("out", [JL, BL], dt.uint8, kind="ExternalOutput")

    with tile.TileContext(nc) as tc:
        with (
            tc.tile_pool(name="const", bufs=1) as constp,
            tc.tile_pool(name="mk", bufs=1) as mkp,
            tc.tile_pool(name="xk", bufs=1) as xkp,
            tc.tile_pool(name="ob", bufs=2) as obp,
        ):
            # warm-up source: zeroed tile, no DMA dependency
            wz = constp.tile([P, 2, BC], dt.uint8)
            nc.vector.memset(wz[:], 0)

            def src2d(t_d, idx):
                return t_d[idx * P:(idx + 1) * P, :].rearrange(
                    "p (ko c) -> p ko c", ko=2)

            # pass-1-critical tiles first on every queue (kp-interleaved)
            mk = {}
            xk = {}
            for jh in range(2):
                for kp in range(KP):
                    mk[(jh, kp)] = mkp.tile([P, 2, JH * P], dt.uint8,
                                            name=f"mk{jh}_{kp}",
                                            tag=f"mk{jh}_{kp}")
            for bc in range(NBC):
                for kp in range(KP):
                    xk[(bc, kp)] = xkp.tile([P, 2, BC], dt.uint8,
                                            name=f"xk{bc}_{kp}",
                                            tag=f"xk{bc}_{kp}")
            # enqueue order: pass-1 tiles (kp-interleaved m/x) round-robin
            # across the three DMA-capable queues, then pass-2 (x bc1) and
            # pass-3 (m jh1) tiles behind them.
            order = []
            for kp in range(KP):
                order.append((mk[(0, kp)], src2d(m_d, kp)))
                order.append((xk[(0, kp)], src2d(x_d, kp)))
            for kp in range(KP):
                order.append((xk[(1, kp)], src2d(x_d, KP + kp)))
            for kp in range(KP):
                order.append((mk[(1, kp)], src2d(m_d, KP + kp)))
            queues = [nc.gpsimd, nc.scalar, nc.sync]
            cth = constp.tile([P, JT], f32)
            cng = constp.tile([P, JT], f32)
            for i, (dst, src) in enumerate(order):
                if i == 2 * KP:  # pass-1 tiles enqueued; consts next
                    nc.scalar.dma_start(cth[:], c_d[:])
                    nc.scalar.dma_start(cng[:], cn_d[:])
                queues[i % 3].dma_start(dst[:], src)

            with tc.tile_pool(name="psacc", bufs=1, space="PSUM") as psacc:
                # PE warm-up: un-throttle HAM while first tiles land
                wps = psacc.tile([P, BC], f32, name="warm", tag="acc1_0")
                for i in range(NWARM):
                    nc.tensor.matmul(
                        wps[:], wz[:, :, 0:P].bitcast(f8), wz[:].bitcast(f8),
                        start=(i == 0), stop=(i == NWARM - 1), perf_mode=DR)

                passes = [(0, 0), (0, 1), (1, 0), (1, 1)]
                for pi, (jh, bc) in enumerate(passes):
                    alt = pi % 2
                    ps = [psacc.tile([P, BC], f32, name=f"acc{pi}_{j4}",
                                     tag=f"acc{alt}_{j4}")
                          for j4 in range(JH)]
                    for kp in range(KP):
                        mt = mk[(jh, kp)]
                        xt = xk[(bc, kp)]
                        for j4 in range(JH):
                            nc.tensor.matmul(
                                ps[j4][:],
                                mt[:, :, j4 * P:(j4 + 1) * P].bitcast(f8),
                                xt[:].bitcast(f8),
                                start=(kp == 0), stop=(kp == KP - 1),
                                perf_mode=DR)
                    for j4 in range(JH):
                        jt = jh * JH + j4
                        ob = obp.tile([P, BC], dt.uint8,
                                      name=f"ob{pi}_{j4}", tag=f"ob{alt}_{j4}")
                        if j4 % 2:
                            # A' and c are both even, so A'-(c+1) is odd:
                            # Sign never sees 0 and the strict compare is
                            # exact; the uint8 cast saturates -1 to 0.
                            nc.scalar.activation(
                                ob[:], ps[j4][:], AF.Sign,
                                bias=cng[:, jt:jt + 1], scale=1.0)
                        else:
                            nc.vector.tensor_scalar(
                                ob[:], ps[j4][:], cth[:, jt:jt + 1], None,
                                op0=ALU.is_gt)
                        oq = nc.gpsimd if j4 % 2 else nc.sync
                        oq.dma_start(
                            o_d[jt * P:(jt + 1) * P, bc * BC:(bc + 1) * BC],
                            ob[:])

    nc.compile()
    return nc


def _get_nc():
    if "nc" not in _cache:
        _cache["nc"] = _build()
    return _cache["nc"]


def _encode_pm1(a01):
    """{0,1} array -> fp8e4 bytes for {-1,+1} (0xB8 / 0x38)."""
    return np.where(a01, np.uint8(0x38), np.uint8(0xB8))


def _tile_k_major(shard):
    """[4096, 1024] byte array -> [KP*128, 1024] with row
    (kp*128 + ki) = [ko=0 512B | ko=1 512B], plus chunk-major stacking
    of the two 512-column halves: returns [2*KP*128, 1024]."""
    t = shard.reshape(KP, 2, P, 2, 512).transpose(3, 0, 2, 1, 4)
    return np.ascontiguousarray(t.reshape(2 * KP * P, 1024))


def run(x, masks, thresholds, trace=False):
    """Run the SPMD kernel on 8 cores. Returns (out_bool, BassKernelResults)."""
    from concourse.bass_utils import run_bass_kernel_spmd

    nc = _get_nc()
    xT8 = np.ascontiguousarray(_encode_pm1(x.T != 0))          # [D, B]
    m8 = _encode_pm1(np.asarray(masks))                        # [D, J]
    cth = (2.0 * thresholds.astype(np.float32) - float(D))     # [J]
    in_maps = []
    for c in range(NCORES):
        bh, jq = c // JS, c % JS
        in_maps.append({
            "xp": _tile_k_major(xT8[:, bh * BL:(bh + 1) * BL]),
            "mp": _tile_k_major(m8[:, jq * JL:(jq + 1) * JL]),
            "cth": np.ascontiguousarray(
                cth[jq * JL:(jq + 1) * JL].reshape(JT, P).T),
            "cng": np.ascontiguousarray(
                -(cth[jq * JL:(jq + 1) * JL] + 1.0).reshape(JT, P).T),
        })
    res = run_bass_kernel_spmd(nc, in_maps, core_ids=list(range(NCORES)),
                               trace=trace)
    out = np.empty((B, J), dtype=np.uint8)
    for c in range(NCORES):
        bh, jq = c // JS, c % JS
        out[bh * BL:(bh + 1) * BL, jq * JL:(jq + 1) * JL] = \
            res.results[c]["out"].T
    return out.view(np.bool_), res


def kernel(x, masks, thresholds):
    x = np.asarray(x)
    masks = np.asarray(masks)
    thresholds = np.asarray(thresholds)
    out, _ = run(x, masks, thresholds, trace=False)
    return out


# revision 11
# speedup vs baseline: 1.0261x; 1.0261x over previous
"""Trainium2 Bass kernel for nn_Block_41077067219413.

Reference computation (B=2048, D=dim_in=4096, J=dim_out=4096):
    xf = x.astype(f32)                 # (B, D) in {0,1}
    mf = masks.astype(f32)             # (D, J) in {0,1}
    sums = xf @ mf + (1-xf) @ (1-mf)   # XNOR popcount over D
    out  = sums > thresholds[None, :]  # (B, J) bool

Identity: with x' = 2x-1 and m' = 2m-1 (both in {-1,+1}),
    A'[j,b] = sum_k m'[k,j] * x'[b,k] = 2*sums - D
    out     = A' > 2*th - D

Both operands are host-encoded as fp8e4 bytes (+1 = 0x38, -1 = 0xB8) and
host-tiled so every DMA row is 2048 contiguous bytes (the DGE descriptor
issue rate, ~12ns/descriptor/queue, is the input-bandwidth limit).  The
device runs one fp8 DoubleRow GEMM per core with no transposes, rowsum,
or threshold folding.  Work is sharded 2 (batch) x 4 (dim_out): each
core computes out_shard [1024 j, 1024 b] = m'^T @ x' with j on PSUM
partitions, so the threshold compare is a per-partition-scalar is_gt
(vector) or Sign activation (scalar), split across both engines.

Schedule: small warm-up matmuls on a zeroed tile un-throttle the PE
clock (HAM) while the first tiles stream in, then 4 passes of (j-half,
b-half), 64 DR matmuls each, alternating two 4-bank PSUM sets so
epilogues overlap the next pass.  Input DMAs round-robin across the
three DMA-capable queues (gpsimd/scalar/sync), pass-1 tiles first.
"""

import numpy as np

B, D, J = 2048, 4096, 4096
NCORES = 8
BS, JS = 2, 4             # batch-shards x j-shards
BL = B // BS              # 1024 batch rows per core
JL = J // JS              # 1024 output cols per core
P = 128
KP = D // 256             # 16 k-pair steps (256 contraction each)
KQ = KP // 2              # 8 fused DMA tiles (2 k-pairs = 512 rows each)
JT = JL // P              # 8 j-tiles of 128
BC = 512                  # batch free-dim chunk (one PSUM bank)
NBC = BL // BC            # 2 batch chunks
JH = JT // 2              # 4 j-tiles per pass
NWARM = 36

_cache = {}


def _build():
    import concourse.bacc as bacc
    import concourse.mybir as mybir
    import concourse.tile as tile

    dt = mybir.dt
    f8 = dt.float8e4
    f32 = dt.float32
    ALU = mybir.AluOpType
    AF = mybir.ActivationFunctionType
    DR = mybir.MatmulPerfMode.DoubleRow

    nc = bacc.Bacc("TRN2", target_bir_lowering=False, debug=False,
                   num_devices=NCORES)

    # host-tiled fp8 bytes; row r = (chunk*KQ + kq)*128 + ki holds 2048
    # contiguous bytes [kpp=0: ko0 512 | ko1 512 | kpp=1: ko0 | ko1]
    x_d = nc.dram_tensor("xp", [NBC * KQ * P, 2048], dt.uint8,
                         kind="ExternalInput")
    m_d = nc.dram_tensor("mp", [2 * KQ * P, 2048], dt.uint8,
                         kind="ExternalInput")
    c_d = nc.dram_tensor("cth", [P, JT], f32, kind="ExternalInput")
    # -(c+1) for the Sign-based epilogue on the Activation engine
    cn_d = nc.dram_tensor("cng", [P, JT], f32, kind="ExternalInput")
    o_d = nc.dram_tensor("out", [JL, BL], dt.uint8, kind="ExternalOutput")

    with tile.TileContext(nc) as tc:
        with (
            tc.tile_pool(name="const", bufs=1) as constp,
            tc.tile_pool(name="mk", bufs=1) as mkp,
            tc.tile_pool(name="xk", bufs=1) as xkp,
            tc.tile_pool(name="ob", bufs=2) as obp,
        ):
            # warm-up source: small zeroed tile, no DMA dependency
            wz = constp.tile([P, 2, 64], dt.uint8)
            nc.vector.memset(wz[:], 0)

            def src2d(t_d, idx):
                return t_d[idx * P:(idx + 1) * P, :].rearrange(
                    "p (kpp ko c) -> p kpp ko c", kpp=2, ko=2)

            mk = {}
            xk = {}
            for jh in range(2):
                for kq in range(KQ):
                    mk[(jh, kq)] = mkp.tile([P, 2, 2, BC], dt.uint8,
                                            name=f"mk{jh}_{kq}",
                                            tag=f"mk{jh}_{kq}")
            for bc in range(NBC):
                for kq in range(KQ):
                    xk[(bc, kq)] = xkp.tile([P, 2, 2, BC], dt.uint8,
                                            name=f"xk{bc}_{kq}",
                                            tag=f"xk{bc}_{kq}")

            # enqueue order: pass-1 tiles (kq-interleaved m/x) round-robin
            # across the three DMA queues, then pass-2 (x bc1) and pass-3
            # (m jh1) tiles
            order = []
            for kq in range(KQ):
                order.append((mk[(0, kq)], src2d(m_d, kq)))
                order.append((xk[(0, kq)], src2d(x_d, kq)))
            for kq in range(KQ):
                order.append((xk[(1, kq)], src2d(x_d, KQ + kq)))
                order.append((mk[(1, kq)], src2d(m_d, KQ + kq)))
            queues = [nc.gpsimd, nc.scalar, nc.sync]
            cth = constp.tile([P, JT], f32)
            cng = constp.tile([P, JT], f32)
            for i, (dst, src) in enumerate(order):
                if i == 2 * KQ:  # pass-1 tiles enqueued; consts next
                    nc.scalar.dma_start(cth[:], c_d[:])
                    nc.scalar.dma_start(cng[:], cn_d[:])
                queues[i % 3].dma_start(dst[:], src)

            with tc.tile_pool(name="psacc", bufs=1, space="PSUM") as psacc:
                # PE warm-up: un-throttle HAM while first tiles land
                wps = psacc.tile([P, BC], f32, name="warm", tag="acc1_0")
                for i in range(NWARM):
                    nc.tensor.matmul(
                        wps[0:32, 0:64], wz[:, :, 0:32].bitcast(f8),
                        wz[:].bitcast(f8),
                        start=(i == 0), stop=(i == NWARM - 1), perf_mode=DR)

                passes = [(0, 0), (0, 1), (1, 0), (1, 1)]
                for pi, (jh, bc) in enumerate(passes):
                    alt = pi % 2
                    ps = [psacc.tile([P, BC], f32, name=f"acc{pi}_{j4}",
                                     tag=f"acc{alt}_{j4}")
                          for j4 in range(JH)]
                    for kp in range(KP):
                        kq, kpp = kp // 2, kp % 2
                        mt = mk[(jh, kq)]
                        xt = xk[(bc, kq)]
                        for j4 in range(JH):
                            nc.tensor.matmul(
                                ps[j4][:],
                                mt[:, kpp, :,
                                   j4 * P:(j4 + 1) * P].bitcast(f8),
                                xt[:, kpp, :, :].bitcast(f8),
                                start=(kp == 0), stop=(kp == KP - 1),
                                perf_mode=DR)
                    for j4 in range(JH):
                        jt = jh * JH + j4
                        ob = obp.tile([P, BC], dt.uint8,
                                      name=f"ob{pi}_{j4}", tag=f"ob{alt}_{j4}")
                        if j4 % 2:
                            # A' and c are both even, so A'-(c+1) is odd:
                            # Sign never sees 0 and the strict compare is
                            # exact; the uint8 cast saturates -1 to 0.
                            nc.scalar.activation(
                                ob[:], ps[j4][:], AF.Sign,
                                bias=cng[:, jt:jt + 1], scale=1.0)
                        else:
                            nc.vector.tensor_scalar(
                                ob[:], ps[j4][:], cth[:, jt:jt + 1], None,
                                op0=ALU.is_gt)
                        oq = nc.gpsimd if j4 % 2 else nc.sync
                        oq.dma_start(
                            o_d[jt * P:(jt + 1) * P, bc * BC:(bc + 1) * BC],
                            ob[:])

    nc.compile()
    return nc


def _get_nc():
    if "nc" not in _cache:
        _cache["nc"] = _build()
    return _cache["nc"]


def _encode_pm1(a01):
    """{0,1} array -> fp8e4 bytes for {-1,+1} (0xB8 / 0x38)."""
    return np.where(a01, np.uint8(0x38), np.uint8(0xB8))


def _tile_k_major(shard):
    """[4096, 1024] byte array (k-major) -> [2*KQ*128, 2048]: row
    (chunk*KQ + kq)*128 + ki = 2048 contiguous bytes covering the two
    k-pairs' ko-interleaved halves of one 512-column chunk."""
    t = shard.reshape(KQ, 2, 2, P, 2, BC)        # [kq, kpp, ko, ki, ch, c]
    t = t.transpose(4, 0, 3, 1, 2, 5)            # [ch, kq, ki, kpp, ko, c]
    return np.ascontiguousarray(t.reshape(2 * KQ * P, 2048))


def run(x, masks, thresholds, trace=False):
    """Run the SPMD kernel on 8 cores. Returns (out_bool, BassKernelResults)."""
    from concourse.bass_utils import run_bass_kernel_spmd

    nc = _get_nc()
    xT8 = np.ascontiguousarray(_encode_pm1(x.T != 0))          # [D, B]
    m8 = _encode_pm1(np.asarray(masks))                        # [D, J]
    cth = (2.0 * thresholds.astype(np.float32) - float(D))     # [J]
    in_maps = []
    for c in range(NCORES):
        bh, jq = c // JS, c % JS
        in_maps.append({
            "xp": _tile_k_major(xT8[:, bh * BL:(bh + 1) * BL]),
            "mp": _tile_k_major(m8[:, jq * JL:(jq + 1) * JL]),
            "cth": np.ascontiguousarray(
                cth[jq * JL:(jq + 1) * JL].reshape(JT, P).T),
            "cng": np.ascontiguousarray(
                -(cth[jq * JL:(jq + 1) * JL] + 1.0).reshape(JT, P).T),
        })
    res = run_bass_kernel_spmd(nc, in_maps, core_ids=list(range(NCORES)),
                               trace=trace)
    out = np.empty((B, J), dtype=np.uint8)
    for c in range(NCORES):
        bh, jq = c // JS, c % JS
        out[bh * BL:(bh + 1) * BL, jq * JL:(jq + 1) * JL] = \
            res.results[c]["out"].T
    return out.view(np.bool_), res


def kernel(x, masks, thresholds):
    x = np.asarray(x)
    masks = np.asarray(masks)
    thresholds = np.asarray(thresholds)
    out, _ = run(x, masks, thresholds, trace=False)
    return out


# revision 12
# speedup vs baseline: 1.0514x; 1.0247x over previous
"""Trainium2 Bass kernel for nn_Block_41077067219413.

Reference computation (B=2048, D=dim_in=4096, J=dim_out=4096):
    xf = x.astype(f32)                 # (B, D) in {0,1}
    mf = masks.astype(f32)             # (D, J) in {0,1}
    sums = xf @ mf + (1-xf) @ (1-mf)   # XNOR popcount over D
    out  = sums > thresholds[None, :]  # (B, J) bool

Identity: with x' = 2x-1 and m' = 2m-1 (both in {-1,+1}),
    A'[j,b] = sum_k m'[k,j] * x'[b,k] = 2*sums - D
    out     = A' > 2*th - D

Both operands are host-encoded as fp8e4 bytes (+1 = 0x38, -1 = 0xB8) and
host-tiled so every DMA row is 2048 contiguous bytes (the DGE descriptor
issue rate is the input-bandwidth limit).  The device runs one fp8
DoubleRow GEMM per core with no transposes, rowsum, or threshold
folding.  Work is sharded 2 (batch) x 4 (dim_out): each core computes
out_shard [1024 j, 1024 b] = m'^T @ x' with j on PSUM partitions, so the
threshold compare is a per-partition-scalar is_gt (vector engine) or
Sign activation (scalar engine), split across both.

Schedule: warm-up matmuls on a zeroed tile un-throttle the PE clock
(HAM) while the first tiles stream in, then 4 passes of (j-half,
b-half), 64 DR matmuls each, alternating two 4-bank PSUM sets so
epilogues overlap the next pass.  Input DMAs are tiered (small first
tiles for a fast ramp, 1MB tiles for the later passes) and kept few
enough that DMA-semaphore reuse never targets an in-flight transfer.
"""

import numpy as np

B, D, J = 2048, 4096, 4096
NCORES = 8
BS, JS = 2, 4             # batch-shards x j-shards
BL = B // BS              # 1024 batch rows per core
JL = J // JS              # 1024 output cols per core
P = 128
KP = D // 256             # 16 k-pair steps (256 contraction each)
KQ = KP // 2              # 8 dram row-blocks (2 k-pairs = 2048B rows)
JT = JL // P              # 8 j-tiles of 128
BC = 512                  # batch free-dim chunk (one PSUM bank)
NBC = BL // BC            # 2 batch chunks
JH = JT // 2              # 4 j-tiles per pass
NWARM = 44

_cache = {}


def _build():
    import concourse.bacc as bacc
    import concourse.mybir as mybir
    import concourse.tile as tile

    dt = mybir.dt
    f8 = dt.float8e4
    f32 = dt.float32
    ALU = mybir.AluOpType
    AF = mybir.ActivationFunctionType
    DR = mybir.MatmulPerfMode.DoubleRow

    nc = bacc.Bacc("TRN2", target_bir_lowering=False, debug=False,
                   num_devices=NCORES)

    # host-tiled fp8 bytes; row r = (chunk*KQ + kq)*128 + ki holds 2048
    # contiguous bytes [kpp=0: ko0 512 | ko1 512 | kpp=1: ko0 | ko1]
    x_d = nc.dram_tensor("xp", [NBC * KQ * P, 2048], dt.uint8,
                         kind="ExternalInput")
    m_d = nc.dram_tensor("mp", [2 * KQ * P, 2048], dt.uint8,
                         kind="ExternalInput")
    c_d = nc.dram_tensor("cth", [P, JT], f32, kind="ExternalInput")
    # -(c+1) for the Sign-based epilogue on the Activation engine
    cn_d = nc.dram_tensor("cng", [P, JT], f32, kind="ExternalInput")
    o_d = nc.dram_tensor("out", [JL, BL], dt.uint8, kind="ExternalOutput")

    with tile.TileContext(nc) as tc:
        with (
            tc.tile_pool(name="const", bufs=1) as constp,
            tc.tile_pool(name="mk", bufs=1) as mkp,
            tc.tile_pool(name="xk", bufs=1) as xkp,
            tc.tile_pool(name="ob", bufs=1) as obp,
        ):
            # warm-up source: small zeroed tile, no DMA dependency
            wz = constp.tile([P, 2, 64], dt.uint8)
            nc.vector.memset(wz[:], 0)

            # --- input tiles -------------------------------------------
            # first half (jh0 masks / bc0 x): kp-split tiles for kq0,
            # then per-kq tiles; second half: two 4-kq (1MB) tiles.
            def hk_src(t_d, ch, kp):
                return t_d[ch * KQ * P:(ch * KQ + 1) * P,
                           (kp % 2) * 1024:(kp % 2 + 1) * 1024].rearrange(
                    "p (ko c) -> p ko c", ko=2)

            def kq_src(t_d, ch, kq):
                return t_d[(ch * KQ + kq) * P:(ch * KQ + kq + 1) * P,
                           :].rearrange("p (kpp ko c) -> p kpp ko c",
                                        kpp=2, ko=2)

            def big_src(t_d, ch, f):
                r0 = (ch * KQ + f * 4) * P
                return t_d[r0:r0 + 4 * P, :].rearrange(
                    "(kq p) (kpp ko c) -> p kq kpp ko c",
                    p=P, kpp=2, ko=2)

            hk = {}   # (op, kp) -> [P, 2, BC]        op 0 = mask, 1 = x
            kqt = {}  # (op, kq) -> [P, 2, 2, BC]
            big = {}  # (op, f)  -> [P, 4, 2, 2, BC]
            for op, pool in ((0, mkp), (1, xkp)):
                for kp in range(2):
                    hk[(op, kp)] = pool.tile([P, 2, BC], dt.uint8,
                                             name=f"hk{op}_{kp}")
                for kq in range(1, KQ):
                    kqt[(op, kq)] = pool.tile([P, 2, 2, BC], dt.uint8,
                                              name=f"kq{op}_{kq}")
                for f in range(2):
                    big[(op, f)] = pool.tile([P, 4, 2, 2, BC], dt.uint8,
                                             name=f"big{op}_{f}")

            order = []
            for kp in range(2):
                order.append((hk[(0, kp)], hk_src(m_d, 0, kp)))
                order.append((hk[(1, kp)], hk_src(x_d, 0, kp)))
            for kq in range(1, KQ):
                order.append((kqt[(0, kq)], kq_src(m_d, 0, kq)))
                order.append((kqt[(1, kq)], kq_src(x_d, 0, kq)))
            for f in range(2):
                order.append((big[(1, f)], big_src(x_d, 1, f)))
            for f in range(2):
                order.append((big[(0, f)], big_src(m_d, 1, f)))
            queues = [nc.gpsimd, nc.scalar, nc.sync]
            cth = constp.tile([P, JT], f32)
            cng = constp.tile([P, JT], f32)
            for i, (dst, src) in enumerate(order):
                if i == 18:  # pass-1 tiles enqueued; consts next
                    nc.sync.dma_start(cth[:], c_d[:])
                    nc.sync.dma_start(cng[:], cn_d[:])
                queues[i % 3].dma_start(dst[:], src)

            def mm_lhsT(jh, kp, j4):
                jsl = slice(j4 * P, (j4 + 1) * P)
                if jh == 0:
                    if kp < 2:
                        return hk[(0, kp)][:, :, jsl]
                    return kqt[(0, kp // 2)][:, kp % 2, :, jsl]
                kq = kp // 2
                return big[(0, kq // 4)][:, kq % 4, kp % 2, :, jsl]

            def mm_rhs(bc, kp):
                if bc == 0:
                    if kp < 2:
                        return hk[(1, kp)][:]
                    return kqt[(1, kp // 2)][:, kp % 2, :, :]
                kq = kp // 2
                return big[(1, kq // 4)][:, kq % 4, kp % 2, :, :]

            obs = [obp.tile([P, BL], dt.uint8, name=f"ob{jt}")
                   for jt in range(JT)]

            with tc.tile_pool(name="psacc", bufs=1, space="PSUM") as psacc:
                # PE warm-up: un-throttle HAM while first tiles land
                wps = psacc.tile([P, BC], f32, name="warm", tag="acc1_0")
                for i in range(NWARM):
                    nc.tensor.matmul(
                        wps[0:32, 0:64], wz[:, :, 0:32].bitcast(f8),
                        wz[:].bitcast(f8),
                        start=(i == 0), stop=(i == NWARM - 1), perf_mode=DR)

                passes = [(0, 0), (0, 1), (1, 0), (1, 1)]
                for pi, (jh, bc) in enumerate(passes):
                    alt = pi % 2
                    ps = [psacc.tile([P, BC], f32, name=f"acc{pi}_{j4}",
                                     tag=f"acc{alt}_{j4}")
                          for j4 in range(JH)]
                    for kp in range(KP):
                        for j4 in range(JH):
                            nc.tensor.matmul(
                                ps[j4][:],
                                mm_lhsT(jh, kp, j4).bitcast(f8),
                                mm_rhs(bc, kp).bitcast(f8),
                                start=(kp == 0), stop=(kp == KP - 1),
                                perf_mode=DR)
                    for j4 in range(JH):
                        jt = jh * JH + j4
                        ob = obs[jt]
                        osl = ob[:, bc * BC:(bc + 1) * BC]
                        if j4 % 2:
                            # A' and c are both even, so A'-(c+1) is odd:
                            # Sign never sees 0 and the strict compare is
                            # exact; the uint8 cast saturates -1 to 0.
                            nc.scalar.activation(
                                osl, ps[j4][:], AF.Sign,
                                bias=cng[:, jt:jt + 1], scale=1.0)
                        else:
                            nc.vector.tensor_scalar(
                                osl, ps[j4][:], cth[:, jt:jt + 1], None,
                                op0=ALU.is_gt)
                        if bc == NBC - 1:
                            oq = nc.gpsimd if j4 % 2 else nc.sync
                            oq.dma_start(o_d[jt * P:(jt + 1) * P, :], ob[:])

    nc.compile()
    return nc


def _get_nc():
    if "nc" not in _cache:
        _cache["nc"] = _build()
    return _cache["nc"]


def _encode_pm1(a01):
    """{0,1} array -> fp8e4 bytes for {-1,+1} (0xB8 / 0x38)."""
    return np.where(a01, np.uint8(0x38), np.uint8(0xB8))


def _tile_k_major(shard):
    """[4096, 1024] byte array (k-major) -> [2*KQ*128, 2048]: row
    (chunk*KQ + kq)*128 + ki = 2048 contiguous bytes covering the two
    k-pairs' ko-interleaved halves of one 512-column chunk."""
    t = shard.reshape(KQ, 2, 2, P, 2, BC)        # [kq, kpp, ko, ki, ch, c]
    t = t.transpose(4, 0, 3, 1, 2, 5)            # [ch, kq, ki, kpp, ko, c]
    return np.ascontiguousarray(t.reshape(2 * KQ * P, 2048))


def run(x, masks, thresholds, trace=False):
    """Run the SPMD kernel on 8 cores. Returns (out_bool, BassKernelResults)."""
    from concourse.bass_utils import run_bass_kernel_spmd

    nc = _get_nc()
    xT8 = np.ascontiguousarray(_encode_pm1(x.T != 0))          # [D, B]
    m8 = _encode_pm1(np.asarray(masks))                        # [D, J]
    cth = (2.0 * thresholds.astype(np.float32) - float(D))     # [J]
    in_maps = []
    for c in range(NCORES):
        bh, jq = c // JS, c % JS
        in_maps.append({
            "xp": _tile_k_major(xT8[:, bh * BL:(bh + 1) * BL]),
            "mp": _tile_k_major(m8[:, jq * JL:(jq + 1) * JL]),
            "cth": np.ascontiguousarray(
                cth[jq * JL:(jq + 1) * JL].reshape(JT, P).T),
            "cng": np.ascontiguousarray(
                -(cth[jq * JL:(jq + 1) * JL] + 1.0).reshape(JT, P).T),
        })
    res = run_bass_kernel_spmd(nc, in_maps, core_ids=list(range(NCORES)),
                               trace=trace)
    out = np.empty((B, J), dtype=np.uint8)
    for c in range(NCORES):
        bh, jq = c // JS, c % JS
        out[bh * BL:(bh + 1) * BL, jq * JL:(jq + 1) * JL] = \
            res.results[c]["out"].T
    return out.view(np.bool_), res


def kernel(x, masks, thresholds):
    x = np.asarray(x)
    masks = np.asarray(masks)
    thresholds = np.asarray(thresholds)
    out, _ = run(x, masks, thresholds, trace=False)
    return out
